# revision 2
# baseline (speedup 1.0000x reference)
"""Trainium2 Bass kernel for ParallelLMHeadWithLoRA.

logits = hidden @ W^T + (hidden @ A^T) @ B^T
  hidden [2048, 4096] f32, W [32000, 4096] f32, A [16, 4096], B [32000, 16]

Strategy (8 NeuronCores, tensor-parallel over vocab):
  - Each core owns a 4000-wide vocab slice of W and B.
  - Host pre-transposes/blocks the operands so every DMA is contiguous:
      wtb[vb, p, dc, j] = W[v0 + vb*125 + j, dc*128 + p]   (per-core, 32 vblocks)
      htt = hidden^T [4096, 2048]                           (replicated)
      att = A^T      [4096, 16]                             (replicated)
      btt = B^T slice [16, 4000]                            (per-core)
  - On device (per core): compute logits^T[v, tok] with the PE:
      out[j, t] = sum_dc  wt[128d, 125v].T @ ht[128d, 512t]   (fp32r matmul)
                + bt[16r, 125v].T @ ao[16r, 512t]             (LoRA, K=16)
    where ao[r, t] = sum_dc at[128d, 16r].T @ ht[128d, 512t] is computed
    on device once per token-half.
  - Tokens are processed in 2 halves of 1024 so the hidden^T half stays
    SBUF-resident; W is streamed twice (DMA ~197MB < PE work => PE-bound).
  - fp32r (single-pass fp32 matmul, 1 cycle/row at N>=256): measured
    ~1.5e-4 median rel err for K=4096 -- well within tolerance.
"""

import numpy as np

import concourse.bass as bass
import concourse.mybir as mybir
import concourse.tile as tile
from concourse import bacc
from concourse.bass_utils import run_bass_kernel_spmd

P = 128
N_TOK = 2048
D = 4096
V = 32000
R = 16
NCORES = 8

VC = V // NCORES          # 4000 vocab per core
VBS = 125                 # vocab block (psum partition dim)
VB = VC // VBS            # 32 vocab blocks
DC = D // P               # 32 contraction chunks
NH = 2                    # token halves
TH = N_TOK // NH          # 1024 tokens per half
TB = 2                    # token blocks per half
TBS = TH // TB            # 512 tokens per matmul (fp32 moving max)

F32 = mybir.dt.float32
F32R = mybir.dt.float32r


def build_nc(ht_bufs=34, wt_bufs=2, out_bufs=3, ps_bufs=4):
    nc = bacc.Bacc(None, target_bir_lowering=False, debug=False)

    wtb = nc.dram_tensor("wtb", [VB, P, DC, VBS], F32R, kind="ExternalInput")
    htt = nc.dram_tensor("htt", [D, N_TOK], F32R, kind="ExternalInput")
    att = nc.dram_tensor("att", [D, R], F32R, kind="ExternalInput")
    btt = nc.dram_tensor("btt", [R, VC], F32R, kind="ExternalInput")
    outt = nc.dram_tensor("outt", [VC, N_TOK], F32, kind="ExternalOutput")

    att3 = att.rearrange("(c p) r -> p c r", p=P)

    with tile.TileContext(nc) as tc:
        with (
            tc.tile_pool(name="const", bufs=1) as const,
            tc.tile_pool(name="htp", bufs=ht_bufs) as htp,
            tc.tile_pool(name="wtp", bufs=wt_bufs) as wtp,
            tc.tile_pool(name="outp", bufs=out_bufs) as outp,
            tc.tile_pool(name="psp", bufs=ps_bufs, space="PSUM") as psp,
            tc.tile_pool(name="aops", bufs=2, space="PSUM") as aops,
        ):
            at_t = const.tile([P, DC, R], F32R, name="at_t")
            nc.sync.dma_start(at_t[:], att3)
            bt_t = const.tile([R, VC], F32R, name="bt_t")
            nc.sync.dma_start(bt_t[:], btt[:, :])
            ao_t = const.tile([R, N_TOK], F32R, name="ao_t")

            for h in range(NH):
                t0h = h * TH
                # resident hidden^T half: 32 tiles of [128, 1024]
                ht_tiles = []
                for dc in range(DC):
                    ht_t = htp.tile([P, TH], F32R, name=f"ht_{h}_{dc}", tag="ht")
                    nc.sync.dma_start(
                        ht_t[:], htt[dc * P:(dc + 1) * P, t0h:t0h + TH]
                    )
                    ht_tiles.append(ht_t)

                # LoRA activations for this half: ao[r, t] (K=4096 accum)
                for tb in range(TB):
                    pa = aops.tile([R, TBS], F32, name="pa", tag="pa")
                    for dc in range(DC):
                        nc.tensor.matmul(
                            pa[:],
                            at_t[:, dc, :],
                            ht_tiles[dc][:, tb * TBS:(tb + 1) * TBS],
                            start=(dc == 0),
                            stop=(dc == DC - 1),
                        )
                    nc.vector.tensor_copy(
                        ao_t[:, t0h + tb * TBS:t0h + (tb + 1) * TBS], pa[:]
                    )

                for vb in range(VB):
                    wt_t = wtp.tile([P, DC, VBS], F32R, name="wt_t", tag="wt")
                    nc.sync.dma_start(wt_t[:], wtb[vb, :, :, :])

                    pss = [
                        psp.tile([VBS, TBS], F32, name=f"ps{tb}", tag="ps")
                        for tb in range(TB)
                    ]
                    for dc in range(DC):
                        for tb in range(TB):
                            nc.tensor.matmul(
                                pss[tb][:],
                                wt_t[:, dc, :],
                                ht_tiles[dc][:, tb * TBS:(tb + 1) * TBS],
                                start=(dc == 0),
                                stop=False,
                            )
                    for tb in range(TB):
                        ts0 = t0h + tb * TBS
                        # fold LoRA correction into the same psum group
                        nc.tensor.matmul(
                            pss[tb][:],
                            bt_t[:, vb * VBS:(vb + 1) * VBS],
                            ao_t[:, ts0:ts0 + TBS],
                            start=False,
                            stop=True,
                        )
                        ot = outp.tile([VBS, TBS], F32, name="ot", tag="ot")
                        nc.vector.tensor_copy(ot[:], pss[tb][:])
                        nc.scalar.dma_start(
                            outt[vb * VBS:(vb + 1) * VBS, ts0:ts0 + TBS], ot[:]
                        )
    nc.compile()
    return nc


def _prep_inputs(hidden_states, weight, lora_A, lora_B):
    w = np.ascontiguousarray(weight, dtype=np.float32)
    # [core, vb, j, dc, p] -> [core, vb, p, dc, j]
    wtb_all = np.ascontiguousarray(
        w.reshape(NCORES, VB, VBS, DC, P).transpose(0, 1, 4, 3, 2)
    )
    htt = np.ascontiguousarray(hidden_states.T, dtype=np.float32)
    att = np.ascontiguousarray(lora_A.T, dtype=np.float32)
    btt_all = np.ascontiguousarray(
        lora_B.reshape(NCORES, VC, R).transpose(0, 2, 1), dtype=np.float32
    )
    return [
        {
            "wtb": wtb_all[c],
            "htt": htt,
            "att": att,
            "btt": btt_all[c],
        }
        for c in range(NCORES)
    ]


def run(hidden_states, weight, lora_A, lora_B, trace=False, **run_kwargs):
    in_maps = _prep_inputs(hidden_states, weight, lora_A, lora_B)
    nc = build_nc()
    res = run_bass_kernel_spmd(
        nc, in_maps, core_ids=list(range(NCORES)), trace=trace, **run_kwargs
    )
    out = np.empty((N_TOK, V), dtype=np.float32)
    for c in range(NCORES):
        out[:, c * VC:(c + 1) * VC] = res.results[c]["outt"].T
    return out, res


def kernel(hidden_states, weight, lora_A, lora_B):
    out, _ = run(hidden_states, weight, lora_A, lora_B, trace=False)
    return out


# revision 5
# speedup vs baseline: 1.1138x; 1.1138x over previous
"""Trainium2 Bass kernel for ParallelLMHeadWithLoRA.

logits = hidden @ W^T + (hidden @ A^T) @ B^T
  hidden [2048, 4096] f32, W [32000, 4096] f32, A [16, 4096], B [32000, 16]

Strategy (8 NeuronCores, tensor-parallel over vocab):
  - Each core owns a 4000-wide vocab slice of W and B (sharding hint).
  - Host pre-transposes/blocks the operands so every DMA is contiguous:
      wtb[vb, p, dc, j] = W[v0 + vb*125 + j, dc*128 + p]   (per-core, 32 vblocks)
      htt = hidden^T [4096, 2048]                           (replicated)
      att = A^T      [4096, 16]                             (replicated)
      btt = B^T slice [16, 4000]                            (per-core)
  - On device (per core): compute logits^T[v, tok] with the PE:
      out[j, t] = sum_dc  wt[128d, 125v].T @ ht[128d, Nt]   (matmul)
                + bt[16r, 125v].T @ ao[16r, Nt]             (LoRA, K=16)
    where ao[r, t] = sum_dc at[128d, 16r].T @ ht[128d, Nt] is computed
    on device once per token-half.
  - Tokens are processed in 2 halves of 1024 so the hidden^T half stays
    SBUF-resident; W is streamed twice => PE-bound at ~1 cycle/row.
  - dtype "f16": fp16 operands (1 cyc/row, N=1024 moving, FWL weight
    loads fully hidden). Accuracy: products are exact in fp32 PSUM;
    only input rounding (2^-11) contributes -- ~2e-4 of output scale.
  - dtype "f32r": fp32r operands (single-pass fp32, N<=512, self-loading
    weight per matmul -> ~20% PE overhead). ~2e-4 of scale as well.
"""

import numpy as np

import concourse.bass as bass
import concourse.mybir as mybir
import concourse.tile as tile
from concourse import bacc
from concourse.bass_utils import run_bass_kernel_spmd

P = 128
N_TOK = 2048
D = 4096
V = 32000
R = 16
NCORES = 8

VC = V // NCORES          # 4000 vocab per core
VBS = 125                 # vocab block (psum partition dim)
VB = VC // VBS            # 32 vocab blocks
DC = D // P               # 32 contraction chunks
NH = 2                    # token halves
TH = N_TOK // NH          # 1024 tokens per half

F32 = mybir.dt.float32
F32R = mybir.dt.float32r
F16 = mybir.dt.float16

DTYPE = "f16"             # "f16" or "f32r"


def build_nc(dtype=DTYPE, ht_bufs=None, wt_bufs=2, out_bufs=3, ps_bufs=None):
    mdt = F16 if dtype == "f16" else F32R
    TBS = 512                               # moving free dim per matmul (ISA cap)
    TB = TH // TBS                          # token blocks per half
    if ht_bufs is None:
        ht_bufs = 44 if dtype == "f16" else 34
    if ps_bufs is None:
        ps_bufs = 4

    nc = bacc.Bacc(None, target_bir_lowering=False, debug=False)

    wtb = nc.dram_tensor("wtb", [VB, P, DC, VBS], mdt, kind="ExternalInput")
    htt = nc.dram_tensor("htt", [D, N_TOK], mdt, kind="ExternalInput")
    att = nc.dram_tensor("att", [D, R], mdt, kind="ExternalInput")
    btt = nc.dram_tensor("btt", [R, VC], mdt, kind="ExternalInput")
    outt = nc.dram_tensor("outt", [VC, N_TOK], F32, kind="ExternalOutput")

    att3 = att.rearrange("(c p) r -> p c r", p=P)

    with tile.TileContext(nc) as tc:
        with (
            tc.tile_pool(name="const", bufs=1) as const,
            tc.tile_pool(name="htp", bufs=ht_bufs) as htp,
            tc.tile_pool(name="wtp", bufs=wt_bufs) as wtp,
            tc.tile_pool(name="outp", bufs=out_bufs) as outp,
            tc.tile_pool(name="psp", bufs=ps_bufs, space="PSUM") as psp,
            tc.tile_pool(name="aops", bufs=2, space="PSUM") as aops,
        ):
            at_t = const.tile([P, DC, R], mdt, name="at_t")
            nc.sync.dma_start(at_t[:], att3)
            bt_t = const.tile([R, VC], mdt, name="bt_t")
            nc.sync.dma_start(bt_t[:], btt[:, :])
            ao_t = const.tile([R, N_TOK], mdt, name="ao_t")

            for h in range(NH):
                t0h = h * TH
                # resident hidden^T half: 32 tiles of [128, TH]
                ht_tiles = []
                for dc in range(DC):
                    ht_t = htp.tile([P, TH], mdt, name=f"ht_{h}_{dc}", tag="ht")
                    nc.sync.dma_start(
                        ht_t[:], htt[dc * P:(dc + 1) * P, t0h:t0h + TH]
                    )
                    ht_tiles.append(ht_t)

                # LoRA activations for this half: ao[r, t] (K=4096 accum)
                for tb in range(TB):
                    pa = aops.tile([R, TBS], F32, name="pa", tag="pa")
                    for dc in range(DC):
                        nc.tensor.matmul(
                            pa[:],
                            at_t[:, dc, :],
                            ht_tiles[dc][:, tb * TBS:(tb + 1) * TBS],
                            start=(dc == 0),
                            stop=(dc == DC - 1),
                        )
                    nc.vector.tensor_copy(
                        ao_t[:, t0h + tb * TBS:t0h + (tb + 1) * TBS], pa[:]
                    )

                for vb in range(VB):
                    wt_t = wtp.tile([P, DC, VBS], mdt, name="wt_t", tag="wt")
                    nc.sync.dma_start(wt_t[:], wtb[vb, :, :, :])

                    pss = [
                        psp.tile([VBS, TBS], F32, name=f"ps{tb}", tag="ps")
                        for tb in range(TB)
                    ]
                    for dc in range(DC):
                        for tb in range(TB):
                            nc.tensor.matmul(
                                pss[tb][:],
                                wt_t[:, dc, :],
                                ht_tiles[dc][:, tb * TBS:(tb + 1) * TBS],
                                start=(dc == 0),
                                stop=False,
                            )
                    for tb in range(TB):
                        ts0 = t0h + tb * TBS
                        # fold LoRA correction into the same psum group
                        nc.tensor.matmul(
                            pss[tb][:],
                            bt_t[:, vb * VBS:(vb + 1) * VBS],
                            ao_t[:, ts0:ts0 + TBS],
                            start=False,
                            stop=True,
                        )
                        ot = outp.tile([VBS, TBS], F32, name="ot", tag="ot")
                        nc.vector.tensor_copy(ot[:], pss[tb][:])
                        nc.scalar.dma_start(
                            outt[vb * VBS:(vb + 1) * VBS, ts0:ts0 + TBS], ot[:]
                        )
    nc.compile()
    return nc


def _prep_inputs(hidden_states, weight, lora_A, lora_B, dtype=DTYPE):
    ndt = np.float16 if dtype == "f16" else np.float32
    w = np.asarray(weight, dtype=ndt)
    # [core, vb, j, dc, p] -> [core, vb, p, dc, j]
    wtb_all = np.ascontiguousarray(
        w.reshape(NCORES, VB, VBS, DC, P).transpose(0, 1, 4, 3, 2)
    )
    htt = np.ascontiguousarray(np.asarray(hidden_states, dtype=ndt).T)
    att = np.ascontiguousarray(np.asarray(lora_A, dtype=ndt).T)
    btt_all = np.ascontiguousarray(
        np.asarray(lora_B, dtype=ndt).reshape(NCORES, VC, R).transpose(0, 2, 1)
    )
    return [
        {
            "wtb": wtb_all[c],
            "htt": htt,
            "att": att,
            "btt": btt_all[c],
        }
        for c in range(NCORES)
    ]


def run(hidden_states, weight, lora_A, lora_B, dtype=DTYPE, trace=False,
        **run_kwargs):
    in_maps = _prep_inputs(hidden_states, weight, lora_A, lora_B, dtype)
    nc = build_nc(dtype)
    res = run_bass_kernel_spmd(
        nc, in_maps, core_ids=list(range(NCORES)), trace=trace, **run_kwargs
    )
    out = np.empty((N_TOK, V), dtype=np.float32)
    for c in range(NCORES):
        out[:, c * VC:(c + 1) * VC] = res.results[c]["outt"].T
    return out, res


def kernel(hidden_states, weight, lora_A, lora_B):
    out, _ = run(hidden_states, weight, lora_A, lora_B, trace=False)
    return out


# revision 6
# speedup vs baseline: 1.1243x; 1.0094x over previous
"""Trainium2 Bass kernel for ParallelLMHeadWithLoRA.

logits = hidden @ W^T + (hidden @ A^T) @ B^T
  hidden [2048, 4096] f32, W [32000, 4096] f32, A [16, 4096], B [32000, 16]

Strategy (8 NeuronCores, tensor-parallel over vocab):
  - Each core owns a 4000-wide vocab slice of W and B (sharding hint).
  - Host pre-transposes/blocks the operands so every DMA is contiguous:
      wtb[vb, p, dc, j] = W[v0 + vb*125 + j, dc*128 + p]   (per-core, 32 vblocks)
      htt = hidden^T [4096, 2048]                           (replicated)
      att = A^T      [4096, 16]                             (replicated)
      btt = B^T slice [16, 4000]                            (per-core)
  - On device (per core): compute logits^T[v, tok] with the PE:
      out[j, t] = sum_dc  wt[128d, 125v].T @ ht[128d, Nt]   (matmul)
                + bt[16r, 125v].T @ ao[16r, Nt]             (LoRA, K=16)
    where ao[r, t] = sum_dc at[128d, 16r].T @ ht[128d, Nt] is computed
    on device once per token-half.
  - Tokens are processed in 2 halves of 1024 so the hidden^T half stays
    SBUF-resident; W is streamed twice => PE-bound at ~1 cycle/row.
  - dtype "f16": fp16 operands (1 cyc/row, N=1024 moving, FWL weight
    loads fully hidden). Accuracy: products are exact in fp32 PSUM;
    only input rounding (2^-11) contributes -- ~2e-4 of output scale.
  - dtype "f32r": fp32r operands (single-pass fp32, N<=512, self-loading
    weight per matmul -> ~20% PE overhead). ~2e-4 of scale as well.
"""

import numpy as np

import concourse.bass as bass
import concourse.mybir as mybir
import concourse.tile as tile
from concourse import bacc
from concourse.bass_utils import run_bass_kernel_spmd

P = 128
N_TOK = 2048
D = 4096
V = 32000
R = 16
NCORES = 8

VC = V // NCORES          # 4000 vocab per core
VBS = 125                 # vocab block (psum partition dim)
VB = VC // VBS            # 32 vocab blocks
DC = D // P               # 32 contraction chunks
NH = 2                    # token halves
TH = N_TOK // NH          # 1024 tokens per half

F32 = mybir.dt.float32
F32R = mybir.dt.float32r
F16 = mybir.dt.float16

DTYPE = "f16"             # "f16" or "f32r"


def build_nc(dtype=DTYPE, ht_bufs=None, wt_bufs=3, out_bufs=3, ps_bufs=None):
    mdt = F16 if dtype == "f16" else F32R
    TBS = 512                               # moving free dim per matmul (ISA cap)
    TB = TH // TBS                          # token blocks per half
    if ht_bufs is None:
        ht_bufs = 64 if dtype == "f16" else 34
    if ps_bufs is None:
        ps_bufs = 4

    nc = bacc.Bacc(None, target_bir_lowering=False, debug=False)

    wtb = nc.dram_tensor("wtb", [VB, P, DC, VBS], mdt, kind="ExternalInput")
    htt = nc.dram_tensor("htt", [D, N_TOK], mdt, kind="ExternalInput")
    att = nc.dram_tensor("att", [D, R], mdt, kind="ExternalInput")
    btt = nc.dram_tensor("btt", [R, VC], mdt, kind="ExternalInput")
    outt = nc.dram_tensor("outt", [VC, N_TOK], F32, kind="ExternalOutput")

    att3 = att.rearrange("(c p) r -> p c r", p=P)

    with tile.TileContext(nc) as tc:
        with (
            tc.tile_pool(name="const", bufs=1) as const,
            tc.tile_pool(name="htp", bufs=ht_bufs) as htp,
            tc.tile_pool(name="wtp", bufs=wt_bufs) as wtp,
            tc.tile_pool(name="outp", bufs=out_bufs) as outp,
            tc.tile_pool(name="psp", bufs=ps_bufs, space="PSUM") as psp,
            tc.tile_pool(name="aops", bufs=2, space="PSUM") as aops,
        ):
            at_t = const.tile([P, DC, R], mdt, name="at_t")
            nc.sync.dma_start(at_t[:], att3)
            bt_t = const.tile([R, VC], mdt, name="bt_t")
            nc.gpsimd.dma_start(bt_t[:], btt[:, :])
            ao_t = const.tile([R, N_TOK], mdt, name="ao_t")

            for h in range(NH):
                t0h = h * TH
                # resident hidden^T half: 32 tiles of [128, TH]
                ht_tiles = []
                for dc in range(DC):
                    ht_t = htp.tile([P, TH], mdt, name=f"ht_{h}_{dc}", tag="ht")
                    nc.sync.dma_start(
                        ht_t[:], htt[dc * P:(dc + 1) * P, t0h:t0h + TH]
                    )
                    ht_tiles.append(ht_t)

                # LoRA activations for this half: ao[r, t] (K=4096 accum)
                for tb in range(TB):
                    pa = aops.tile([R, TBS], F32, name="pa", tag="pa")
                    for dc in range(DC):
                        nc.tensor.matmul(
                            pa[:],
                            at_t[:, dc, :],
                            ht_tiles[dc][:, tb * TBS:(tb + 1) * TBS],
                            start=(dc == 0),
                            stop=(dc == DC - 1),
                        )
                    nc.vector.tensor_copy(
                        ao_t[:, t0h + tb * TBS:t0h + (tb + 1) * TBS], pa[:]
                    )

                for vb in range(VB):
                    wt_t = wtp.tile([P, DC, VBS], mdt, name="wt_t", tag="wt")
                    nc.sync.dma_start(wt_t[:], wtb[vb, :, :, :])

                    pss = [
                        psp.tile([VBS, TBS], F32, name=f"ps{tb}", tag="ps")
                        for tb in range(TB)
                    ]
                    for dc in range(DC):
                        for tb in range(TB):
                            nc.tensor.matmul(
                                pss[tb][:],
                                wt_t[:, dc, :],
                                ht_tiles[dc][:, tb * TBS:(tb + 1) * TBS],
                                start=(dc == 0),
                                stop=False,
                            )
                    for tb in range(TB):
                        ts0 = t0h + tb * TBS
                        # fold LoRA correction into the same psum group
                        nc.tensor.matmul(
                            pss[tb][:],
                            bt_t[:, vb * VBS:(vb + 1) * VBS],
                            ao_t[:, ts0:ts0 + TBS],
                            start=False,
                            stop=True,
                        )
                        ot = outp.tile([VBS, TBS], F32, name="ot", tag="ot")
                        nc.vector.tensor_copy(ot[:], pss[tb][:])
                        nc.scalar.dma_start(
                            outt[vb * VBS:(vb + 1) * VBS, ts0:ts0 + TBS], ot[:]
                        )
    nc.compile()
    return nc


def _prep_inputs(hidden_states, weight, lora_A, lora_B, dtype=DTYPE):
    ndt = np.float16 if dtype == "f16" else np.float32
    w = np.asarray(weight, dtype=ndt)
    # [core, vb, j, dc, p] -> [core, vb, p, dc, j]
    wtb_all = np.ascontiguousarray(
        w.reshape(NCORES, VB, VBS, DC, P).transpose(0, 1, 4, 3, 2)
    )
    htt = np.ascontiguousarray(np.asarray(hidden_states, dtype=ndt).T)
    att = np.ascontiguousarray(np.asarray(lora_A, dtype=ndt).T)
    btt_all = np.ascontiguousarray(
        np.asarray(lora_B, dtype=ndt).reshape(NCORES, VC, R).transpose(0, 2, 1)
    )
    return [
        {
            "wtb": wtb_all[c],
            "htt": htt,
            "att": att,
            "btt": btt_all[c],
        }
        for c in range(NCORES)
    ]


def run(hidden_states, weight, lora_A, lora_B, dtype=DTYPE, trace=False,
        **run_kwargs):
    in_maps = _prep_inputs(hidden_states, weight, lora_A, lora_B, dtype)
    nc = build_nc(dtype)
    res = run_bass_kernel_spmd(
        nc, in_maps, core_ids=list(range(NCORES)), trace=trace, **run_kwargs
    )
    out = np.empty((N_TOK, V), dtype=np.float32)
    for c in range(NCORES):
        out[:, c * VC:(c + 1) * VC] = res.results[c]["outt"].T
    return out, res


def kernel(hidden_states, weight, lora_A, lora_B):
    out, _ = run(hidden_states, weight, lora_A, lora_B, trace=False)
    return out
